# revision 8
# baseline (speedup 1.0000x reference)
"""Trainium2 Bass kernel for causal MultiHeadAttention (B=2, S=2048, E=1024, H=16).

Sharding: 8 cores = 2 (batch) x 4 (head groups of 4, Megatron-style).
Each core computes, for its batch b and head group g:
  - Q/K projections into transposed layout qhT/khT [256, S]  (256 = 4 heads x 64)
  - V projection into natural layout vh [S, 256] with a ones-column per head
  - causal attention with scores kept transposed [k, q]; softmax denominators
    come out of the PV matmul via the ones-column; no max-subtraction needed
    (|scores/sqrt(D)| <~ 6 so exp is well within fp32 range; masked entries are
    zeroed AFTER exp, which matches the reference's -1e9 masking exactly)
  - partial output projection attn_concat @ Wo[rows of g]  -> [S, E]
Host sums the 4 partials per batch and adds bo.

All matmul operands are float16 (full PE rate, fp32 PSUM accumulation).
Schedule notes (v2):
  - All input DMAs are issued from the gpsimd (Pool/SWDGE) queue: SP-queue
    DMA dispatch costs ~800ns each and serialized the prologue (~12us of PE
    idle at startup); gpsimd dispatch is ~25ns.
  - Causal masks are built on-device (memset + affine_select) and the ones
    tiles via memset, removing 0.55MB of prologue DMA.
  - Q is processed in variable-size rounds (512,512,512,256,256): the final
    round's serial tail (softmax-denominator chain -> Wo -> cast -> DMA out)
    only covers 256 q rows, and round 3's tail hides under round 4's
    attention matmuls.
  - Projection/output-projection matmuls interleave into the attention
    stream as PE filler, softmax denominators ride a ones-column in the PV
    matmul, and 1/sum is computed as exp(-ln(sum)) on the Scalar engine.
"""

import numpy as np

B, S, E, H = 2, 2048, 1024, 16
D = E // H            # 64 head dim
HL = 4                # heads per core
CW = HL * D           # 256 local channels
P = 128
KT = E // P           # 8 contraction tiles for the input projections
D1 = D + 1            # head slot in vh (+ ones column)
NQMAX = 512

ROUNDS = [(0, 512), (512, 512), (1024, 512), (1536, 256), (1792, 256)]
# round r -> list of earlier rounds whose output projection runs as filler
WO_SCHED = {2: [0], 3: [1], 4: [2, 3]}

_CACHE = {}


def _pin_act_table(mybir, bacc):
    """Force all activations onto one LUT set containing exp+ln+identity, so
    the ACT engine never reloads tables mid-kernel (1.3us per reload)."""
    from concourse.hw_specs import get_activation_tables

    need = {
        mybir.ActivationFunctionType.Exp,
        mybir.ActivationFunctionType.Ln,
        mybir.ActivationFunctionType.Identity,
    }
    orig = get_activation_tables("gen3")
    target = next(n for n, fs in orig.items() if need <= fs)
    pinned = {n: (fs if n == target else set()) for n, fs in orig.items()}
    bacc.get_activation_tables = lambda arch: pinned


def _build(num_devices=8):
    import concourse.mybir as mybir
    import concourse.tile as tile
    from concourse import bacc

    _pin_act_table(mybir, bacc)

    f32 = mybir.dt.float32
    h16 = mybir.dt.float16
    Ln = mybir.ActivationFunctionType.Ln
    Exp = mybir.ActivationFunctionType.Exp

    nc = bacc.Bacc(
        "TRN2", target_bir_lowering=False, debug=False, num_devices=num_devices
    )

    def din(name, shape, dt=f32):
        return nc.dram_tensor(name, list(shape), dt, kind="ExternalInput").ap()

    xqt = din("xqt", (E, S), h16)
    xkt = din("xkt", (E, S), h16)
    xvt = din("xvt", (E, S), h16)
    wq = din("wq", (E, CW), h16)
    wk = din("wk", (E, CW), h16)
    wv = din("wv", (E, CW), h16)
    wo = din("wo", (CW, E), h16)
    bq = din("bq", (CW,))
    bk = din("bk", (CW,))
    bv = din("bv", (CW,), h16)
    masks = din("masks", (P, 4 * NQMAX), h16)
    out = nc.dram_tensor("out", [S, E], h16, kind="ExternalOutput").ap()

    SB = S // P  # 16 k-blocks total

    with tile.TileContext(nc) as tc:
        with (
            tc.tile_pool(name="singles", bufs=1) as singles,
            tc.tile_pool(name="xpool", bufs=6) as xpool,
            tc.tile_pool(name="exp", bufs=10) as exp_pool,
            tc.tile_pool(name="outp", bufs=4) as out_pool,
            tc.tile_pool(name="small", bufs=4) as small_pool,
            tc.tile_pool(name="stage", bufs=6) as stage_pool,
            tc.tile_pool(name="proj_ps", bufs=2, space="PSUM") as proj_ps,
            tc.tile_pool(name="scores_ps", bufs=2, space="PSUM") as scores_ps,
            tc.tile_pool(name="attn_ps", bufs=2, space="PSUM") as attn_ps,
        ):
            dma = nc.sync.dma_start       # input loads
            dma_out = nc.sync.dma_start   # output stores: SP HWDGE

            # --- persistent SBUF tensors -------------------------------------
            wq_sb = singles.tile([P, KT, CW], h16, tag="wq")
            wk_sb = singles.tile([P, KT, CW], h16, tag="wk")
            wv_sb = singles.tile([P, KT, CW], h16, tag="wv")
            wo_sb = singles.tile([P, CW // P, E], h16, tag="wo")
            masks_sb = singles.tile([P, 4, NQMAX], h16, tag="masks")
            bq_sb = singles.tile([P, 2], f32, tag="bq")
            bk_sb = singles.tile([P, 2], f32, tag="bk")
            bv_row = singles.tile([1, CW], h16, tag="bv")
            ones_col = singles.tile([1, P], h16, tag="ones")

            qhT = [singles.tile([P, S], h16, name=f"qhT{m}", tag=f"qhT{m}") for m in range(2)]
            khT = [singles.tile([P, S], h16, name=f"khT{m}", tag=f"khT{m}") for m in range(2)]
            atT = [singles.tile([P, S], h16, name=f"atT{m}", tag=f"atT{m}") for m in range(2)]
            vh = singles.tile([P, SB, HL, D1], h16, tag="vh")

            def t_wk():
                rw = wk.rearrange("(kt p) m -> p kt m", p=P)
                dma(out=wk_sb[:, :1, :], in_=rw[:, :1, :])
                dma(out=wk_sb[:, 1 : KT // 2, :], in_=rw[:, 1 : KT // 2, :])
                dma(out=wk_sb[:, KT // 2 :, :], in_=rw[:, KT // 2 :, :])
                dma(out=bk_sb, in_=bk.rearrange("(m p) -> p m", p=P))

            def t_wv():
                dma(out=wv_sb, in_=wv.rearrange("(kt p) m -> p kt m", p=P))
                dma(out=bv_row, in_=bv.unsqueeze(0))
                # ones for the v-bias rank-1 matmul (read by proj_v tails)
                nc.gpsimd.memset(ones_col, 1.0)

            def t_wq():
                dma(out=wq_sb, in_=wq.rearrange("(kt p) m -> p kt m", p=P))
                dma(out=bq_sb, in_=bq.rearrange("(m p) -> p m", p=P))

            def t_consts():
                # vh's denominator ones-column
                nc.vector.memset(vh[:, :, :, D:D1], 1.0)
                # causal masks: masks_sb[p, jj, q] = (q >= p + 128*jj)
                dma(
                    out=masks_sb, in_=masks.rearrange("p (j n) -> p j n", n=NQMAX)
                )

            def t_wo():
                dma(out=wo_sb, in_=wo.rearrange("(kt p) n -> p kt n", p=P))

            # --- stage helpers (thunk-list builders) -------------------------
            def load_x_thunk(src, qa, nq, holder, key, fine=False):
                def t():
                    tl = xpool.tile([P, KT, NQMAX], h16, name="xchunk", tag="xchunk")
                    rsrc = src.rearrange("(kt p) s -> p kt s", p=P)[
                        :, :, qa : qa + nq
                    ]
                    h = KT // 2
                    if fine:
                        dma(out=tl[:, :1, :nq], in_=rsrc[:, :1, :])
                        dma(out=tl[:, 1:h, :nq], in_=rsrc[:, 1:h, :])
                    else:
                        dma(out=tl[:, :h, :nq], in_=rsrc[:, :h, :])
                    dma(out=tl[:, h:, :nq], in_=rsrc[:, h:, :])
                    holder[key] = tl
                return [t]

            def proj_qk_thunks(qa, nq, holder, key, w_sb, b_sb, dstT):
                thunks = []
                pss = {}
                for m in range(2):
                    def mk_mm(m, kt):
                        def t():
                            if kt == 0:
                                pss[m] = proj_ps.tile([P, NQMAX], f32, name="proj", tag="proj")
                            nc.tensor.matmul(
                                pss[m][:, :nq],
                                w_sb[:, kt, m * P : (m + 1) * P],
                                holder[key][:, kt, :nq],
                                start=(kt == 0),
                                stop=(kt == KT - 1),
                            )
                        return t
                    for kt in range(KT):
                        thunks.append(mk_mm(m, kt))
                    def mk_copy(m):
                        def t():
                            nc.vector.tensor_scalar_add(
                                out=dstT[m][:, qa : qa + nq],
                                in0=pss[m][:, :nq],
                                scalar1=b_sb[:, m : m + 1],
                            )
                        return t
                    thunks.append(mk_copy(m))
                return thunks

            def proj_v_thunks(qa, nq, holder, key):
                thunks = []
                pss = {}
                for mb in range(nq // P):
                    j = qa // P + mb
                    def mk_mm(mb, kt):
                        def t():
                            if kt == 0:
                                pss[mb] = proj_ps.tile([P, NQMAX], f32, name="proj", tag="proj")
                            nc.tensor.matmul(
                                pss[mb][:, :CW],
                                holder[key][:, kt, mb * P : (mb + 1) * P],
                                wv_sb[:, kt, :],
                                start=(kt == 0),
                                stop=False,
                            )
                        return t
                    for kt in range(KT):
                        thunks.append(mk_mm(mb, kt))
                    def mk_tail(mb, j):
                        def t():
                            nc.tensor.matmul(
                                pss[mb][:, :CW],
                                ones_col,
                                bv_row,
                                start=False,
                                stop=True,
                            )
                            nc.vector.tensor_copy(
                                out=vh[:, j, :, 0:D],
                                in_=pss[mb][:, :CW].rearrange("p (h d) -> p h d", h=HL),
                            )
                        return t
                    thunks.append(mk_tail(mb, j))
                return thunks

            def attn_thunks(qa, nq):
                thunks = []
                cbase = qa // P
                nblk = (qa + nq) // P
                scale = float(1.0 / np.sqrt(D))
                for hp in range(2):
                    ats = {}
                    def mk_j(hp, j, ats):
                        def t():
                            if j == 0:
                                ats[0] = attn_ps.tile([D1, NQMAX], f32, name="attn", tag="attn")
                                ats[1] = attn_ps.tile([D1, NQMAX], f32, name="attn", tag="attn")
                            jj = j - cbase
                            q0 = jj * P if jj > 0 else 0
                            sc2 = scores_ps.tile([P, 2, NQMAX], f32, name="sc2", tag="sc2")
                            for hh in range(2):
                                po = hh * D
                                nc.tensor.matmul(
                                    sc2[:, hh, q0:nq],
                                    khT[hp][po : po + D, j * P : (j + 1) * P],
                                    qhT[hp][po : po + D, qa + q0 : qa + nq],
                                    start=True,
                                    stop=True,
                                )
                            ex2 = exp_pool.tile([P, 2, NQMAX], h16, name="ex2", tag="ex2")
                            nc.scalar.activation(
                                out=ex2[:, :, q0:nq], in_=sc2[:, :, q0:nq], func=Exp,
                                scale=scale,
                            )
                            if jj >= 0:
                                for hh in range(2):
                                    exh = ex2[:, hh, q0:nq]
                                    nc.vector.tensor_mul(
                                        exh, exh, masks_sb[:, jj, q0:nq]
                                    )
                            for hh in range(2):
                                nc.tensor.matmul(
                                    ats[hh][:, q0:nq],
                                    vh[:, j, 2 * hp + hh, :],
                                    ex2[:, hh, q0:nq],
                                    start=(j == 0),
                                    stop=(j == nblk - 1),
                                )
                        return t
                    for j in range(nblk):
                        thunks.append(mk_j(hp, j, ats))

                    atu = {}
                    lns = {}
                    def mk_stage(hh, ats, atu, lns):
                        def t():
                            ls = small_pool.tile([1, NQMAX], f32, name="ls", tag="ls")
                            nc.scalar.activation(
                                out=ls[:, :nq], in_=ats[hh][D : D + 1, :nq], func=Ln,
                                scale=1.0,
                            )
                            lns[hh] = ls
                            atu[hh] = stage_pool.tile(
                                [D, NQMAX], h16, name="atu", tag="atu"
                            )
                            nc.vector.tensor_copy(atu[hh][:, :nq], ats[hh][0:D, :nq])
                        return t
                    thunks.append(mk_stage(0, ats, atu, lns))
                    thunks.append(mk_stage(1, ats, atu, lns))

                    def mk_norm(hp, hh, atu, lns):
                        def t():
                            po = hh * D
                            rs = small_pool.tile([1, NQMAX], f32, name="rs", tag="rs")
                            nc.scalar.activation(
                                out=rs[:, :nq], in_=lns[hh][:, :nq], func=Exp, scale=-1.0
                            )
                            rb = small_pool.tile([D, NQMAX], f32, name="rb", tag="rb")
                            nc.gpsimd.partition_broadcast(rb[:, :nq], rs[:, :nq])
                            nc.vector.tensor_mul(
                                atT[hp][po : po + D, qa : qa + nq],
                                atu[hh][:, :nq],
                                rb[:, :nq],
                            )
                        return t
                    thunks.append(mk_norm(hp, 0, atu, lns))
                    thunks.append(mk_norm(hp, 1, atu, lns))
                return thunks

            def wo_thunks(qa, nq):
                thunks = []
                for mb in range(nq // P):
                    ms = qa // P + mb
                    for n in range(2):
                        def mk(ms, n):
                            def t():
                                ps = proj_ps.tile([P, NQMAX], f32, name="proj", tag="proj")
                                for kt in range(CW // P):
                                    nc.tensor.matmul(
                                        ps,
                                        atT[kt][:, ms * P : (ms + 1) * P],
                                        wo_sb[:, kt, n * NQMAX : (n + 1) * NQMAX],
                                        start=(kt == 0),
                                        stop=(kt == CW // P - 1),
                                    )
                                ot = out_pool.tile([P, NQMAX], h16, name="ot", tag="ot")
                                nc.vector.tensor_copy(ot, ps)
                                dma_out(
                                    out=out[
                                        ms * P : (ms + 1) * P, n * NQMAX : (n + 1) * NQMAX
                                    ],
                                    in_=ot,
                                )
                            return t
                        thunks.append(mk(ms, n))
                return thunks

            def wo_tail_thunks(qa, nq):
                """Final-round wo, kt-split: kt=0 (pair-0 atT, ready early)
                issues during pair-1's norm chain; concurrent PSUM groups
                (2 proj + borrowed scores banks)."""
                thunks = []
                units = [(qa // P + mb, n) for mb in range(nq // P) for n in range(2)]
                for wave in (units[:4], units[4:]):
                    if not wave:
                        continue
                    pss = {}
                    def mk_kt0(i, ms, n, pss):
                        def t():
                            if i < 2:
                                pss[i] = proj_ps.tile(
                                    [P, NQMAX], f32, name="proj", tag="proj"
                                )
                            else:
                                ps2 = scores_ps.tile(
                                    [P, 2, NQMAX], f32, name="sc2", tag="sc2"
                                )
                                pss[i] = ps2[:, 0, :]
                            nc.tensor.matmul(
                                pss[i],
                                atT[0][:, ms * P : (ms + 1) * P],
                                wo_sb[:, 0, n * NQMAX : (n + 1) * NQMAX],
                                start=True,
                                stop=False,
                            )
                        return t
                    def mk_kt1(i, ms, n, pss):
                        def t():
                            nc.tensor.matmul(
                                pss[i],
                                atT[1][:, ms * P : (ms + 1) * P],
                                wo_sb[:, 1, n * NQMAX : (n + 1) * NQMAX],
                                start=False,
                                stop=True,
                            )
                            ot = out_pool.tile([P, NQMAX], h16, name="ot", tag="ot")
                            nc.vector.tensor_copy(ot, pss[i])
                            dma_out(
                                out=out[ms * P : (ms + 1) * P, n * NQMAX : (n + 1) * NQMAX],
                                in_=ot,
                            )
                        return t
                    for i, (ms, n) in enumerate(wave):
                        thunks.append(mk_kt0(i, ms, n, pss))
                    for i, (ms, n) in enumerate(wave):
                        thunks.append(mk_kt1(i, ms, n, pss))
                return thunks

            def emit_interleaved(primary, filler):
                fi = 0
                n = max(len(primary), 1)
                f = len(filler)
                for i, t in enumerate(primary):
                    t()
                    while fi * n < f * (i + 1):
                        filler[fi]()
                        fi += 1
                for t in filler[fi:]:
                    t()

            # --- main schedule ----------------------------------------------
            holder = {}
            qa0, nq0 = ROUNDS[0]
            prologue = (
                [t_wk]
                + load_x_thunk(xkt, qa0, nq0, holder, ("xk", 0), fine=True)
                + proj_qk_thunks(qa0, nq0, holder, ("xk", 0), wk_sb, bk_sb, khT)
                + [t_wv]
                + load_x_thunk(xvt, qa0, nq0, holder, ("xv", 0), fine=True)
                + proj_v_thunks(qa0, nq0, holder, ("xv", 0))
                + [t_wq]
                + load_x_thunk(xqt, qa0, nq0, holder, ("xq", 0), fine=True)
                + proj_qk_thunks(qa0, nq0, holder, ("xq", 0), wq_sb, bq_sb, qhT)
                + [t_consts]
            )
            for t in prologue:
                t()
            kv_deferred = {}
            NR = len(ROUNDS)
            for r, (qa, nq) in enumerate(ROUNDS):
                kv_filler = kv_deferred.pop(r, [])
                filler = []
                if r == 0:
                    filler += [t_wo]
                for rr in WO_SCHED.get(r, []):
                    filler += wo_thunks(*ROUNDS[rr])
                if r + 1 < NR:
                    qn, nn = ROUNDS[r + 1]
                    filler += load_x_thunk(xkt, qn, nn, holder, ("xk", r + 1))
                    filler += load_x_thunk(xvt, qn, nn, holder, ("xv", r + 1))
                    filler += load_x_thunk(xqt, qn, nn, holder, ("xq", r + 1))
                    filler += proj_qk_thunks(
                        qn, nn, holder, ("xq", r + 1), wq_sb, bq_sb, qhT
                    )
                    filler += proj_qk_thunks(
                        qn, nn, holder, ("xk", r + 1), wk_sb, bk_sb, khT
                    )
                    kv_deferred[r + 1] = proj_v_thunks(qn, nn, holder, ("xv", r + 1))
                att = attn_thunks(qa, nq)
                cbase = qa // P
                seg1, seg2 = att[:cbase], att[cbase:]
                emit_interleaved(seg1, kv_filler)
                # hold back a quarter of the filler to keep PE fed through the
                # end-of-round normalization chains
                cut = (3 * len(filler)) // 4
                emit_interleaved(seg2[:-8], filler[:cut])
                emit_interleaved(seg2[-8:], filler[cut:])
            for t in wo_tail_thunks(*ROUNDS[-1]):
                t()

    nc.compile()
    return nc


def _get_nc():
    if "nc" not in _CACHE:
        _CACHE["nc"] = _build()
    return _CACHE["nc"]


def make_masks():
    m = np.zeros((P, 4, NQMAX), np.float32)
    ql = np.arange(NQMAX)[None, :]
    kl = np.arange(P)[:, None]
    for jj in range(4):
        m[:, jj, :] = (ql >= kl + jj * P).astype(np.float32)
    return m.reshape(P, 4 * NQMAX)


def make_in_maps(q, k, v, Wq, bq, Wk, bk, Wv, bv, Wo):
    masks = make_masks()
    in_maps = []
    for core in range(8):
        b, g = divmod(core, 4)
        cs = slice(g * CW, (g + 1) * CW)
        in_maps.append(
            {
                "xqt": np.ascontiguousarray(q[b].T).astype(np.float16),
                "xkt": np.ascontiguousarray(k[b].T).astype(np.float16),
                "xvt": np.ascontiguousarray(v[b].T).astype(np.float16),
                "wq": np.ascontiguousarray(Wq[:, cs]).astype(np.float16),
                "wk": np.ascontiguousarray(Wk[:, cs]).astype(np.float16),
                "wv": np.ascontiguousarray(Wv[:, cs]).astype(np.float16),
                "wo": np.ascontiguousarray(Wo[cs, :]).astype(np.float16),
                "bq": np.ascontiguousarray(bq[cs]),
                "bk": np.ascontiguousarray(bk[cs]),
                "bv": np.ascontiguousarray(bv[cs]).astype(np.float16),
                "masks": masks.astype(np.float16),
            }
        )
    return in_maps


def run(q, k, v, Wq, bq, Wk, bk, Wv, bv, Wo, bo, **run_kwargs):
    """Returns (output, BassKernelResults)."""
    from concourse.bass_utils import run_bass_kernel_spmd

    q, k, v = (np.asarray(x, np.float32) for x in (q, k, v))
    nc = _get_nc()
    in_maps = make_in_maps(
        q, k, v,
        np.asarray(Wq, np.float32), np.asarray(bq, np.float32),
        np.asarray(Wk, np.float32), np.asarray(bk, np.float32),
        np.asarray(Wv, np.float32), np.asarray(bv, np.float32),
        np.asarray(Wo, np.float32),
    )
    res = run_bass_kernel_spmd(nc, in_maps, list(range(8)), **run_kwargs)
    out = np.zeros((B, S, E), np.float32)
    for core in range(8):
        out[core // 4] += res.results[core]["out"].astype(np.float32)
    out += np.asarray(bo, np.float32)[None, None, :]
    return out, res


def kernel(q, k, v, Wq, bq, Wk, bk, Wv, bv, Wo, bo):
    return run(q, k, v, Wq, bq, Wk, bk, Wv, bv, Wo, bo)[0]
